# revision 6
# baseline (speedup 1.0000x reference)
"""AGSM Trainium2 kernel: attention-gated temporal shift module on 8 NeuronCores.

Sharding: data-parallel over clips. B=16 clips; core k handles clips (2k, 2k+1).
BN batch stats via tiny AllReduce (or local-BN per shard behind a flag).

Per-core layout (per frame u, 128 partitions): p = 64*clip + q where partition
q holds input channel chmap[q] = 32*(q//32) + 16*((q%32)%2) + (q%32)//2, chosen
so that partition order == final interleaved OUTPUT channel order (stores are
then a single contiguous 128-partition DMA per frame).

Convs are factorized into shift-sum matmul stages (all free-axis tap offsets):
  conv2d: [dx: 3 streams, K=128 -> 24 (dy,head,clip)] -> [dy: 3 streams,
          24 -> 128 replicated attn] -> sigmoid
  conv3d: [dx: 3 -> 36 (dt,dy,g,clip)] -> [dy: 3 -> 12 (dt,g,clip)] ->
          [dt: 3 over t-ring -> 128 replicated gate] -> tanh
This cuts PE column traffic ~4.8x vs direct per-tap matmuls.

Spatial planes padded to 58x58 with zero borders so taps are free-axis offsets.
x is host-pre-cast to bf16 + channel-permuted; output leaves as bf16 [8,128,...]
and is cast/unpermuted on host.
"""
import numpy as np
import ml_dtypes

import concourse.bass as bass
import concourse.tile as tile
from concourse import mybir
from concourse.bass_utils import run_bass_kernel_spmd

N_CORES = 8
T = 8
PS = 3364            # padded plane 58*58
IN = 3136            # interior 56*56
ROWG = 58            # guard for row shifts (s2/s3 tiles)
PG = 59              # guard for X/bnr plane tiles
QUAR = 841
EPS = 1e-5
LOCAL_BN = False
NTOT_GLOBAL = 16 * T * IN
NTOT_LOCAL = 2 * T * IN
F32 = mybir.dt.float32
BF16 = mybir.dt.bfloat16
AF = mybir.ActivationFunctionType
ALU = mybir.AluOpType

_CACHE = {}

SUBS = ((0, 512), (512, 329))    # psum bank-safe sub-chunks of a quarter


def _legalize_waits(nc):
    """This walrus accepts <=1 sync wait per instruction (2 for EventSemaphore).
    Hoist excess waits onto fresh same-engine NoOps inserted just before."""
    n = [0]
    for f in nc.m.functions:
        for bb in f.blocks:
            insts = bb.instructions  # live list
            i = 0
            while i < len(insts):
                inst = insts[i]
                si = inst.sync_info
                cap = 2 if type(inst).__name__ == "InstEventSemaphore" else 1
                if si is not None and len(si.on_wait) > cap:
                    waits = list(si.on_wait)
                    si.on_wait = waits[-cap:]
                    inst.sync_info = si
                    for w in waits[:-cap]:
                        n[0] += 1
                        nop = mybir.InstNoOp(
                            name=f"waitfix-{n[0]}", engine=inst.engine,
                            bass_nofuse=True,
                            sync_info=mybir.SyncInfo(on_wait=[w], on_update=[]))
                        nc.register_instruction(nop, overwrite=True)
                        insts.insert(i, nop)
                        i += 1
                i += 1


def build_nc(local_bn=LOCAL_BN):
    nc = bass.Bass(num_devices=N_CORES)
    x_e = nc.declare_dram_parameter("x", [T, 128, 56, 56], BF16, isOutput=False)
    w1_e = nc.declare_dram_parameter("w1", [128, 3 * 24], BF16, isOutput=False)
    d2_e = nc.declare_dram_parameter("d2", [24, 3 * 128], BF16, isOutput=False)
    w3_e = nc.declare_dram_parameter("w3", [128, 3 * 36], BF16, isOutput=False)
    d3a_e = nc.declare_dram_parameter("d3a", [36, 3 * 12], BF16, isOutput=False)
    d3b_e = nc.declare_dram_parameter("d3b", [12, 3 * 128], BF16, isOutput=False)
    b2_e = nc.declare_dram_parameter("b2", [128, 1], F32, isOutput=False)
    b3_e = nc.declare_dram_parameter("b3", [128, 1], F32, isOutput=False)
    gam_e = nc.declare_dram_parameter("gam", [128, 1], F32, isOutput=False)
    bet_e = nc.declare_dram_parameter("bet", [128, 1], F32, isOutput=False)
    out_e = nc.declare_dram_parameter("out", [T, 128, 56, 56], BF16, isOutput=True)

    XLEN = PG + T * PS + PG

    with tile.TileContext(nc) as tc:
        with (
            tc.tile_pool(name="const", bufs=1) as cpool,
            tc.tile_pool(name="xbuf", bufs=1) as xpool,
            tc.tile_pool(name="stg", bufs=2) as spool,      # s2/s3/bnr staging
            tc.tile_pool(name="attn", bufs=2) as apool,
            tc.tile_pool(name="gate", bufs=2) as gpool,
            tc.tile_pool(name="ring", bufs=1) as rpool,     # P ring + t ring
            tc.tile_pool(name="ybuf", bufs=2) as ypool,
            tc.tile_pool(name="psum", bufs=2, space=bass.MemorySpace.PSUM) as psum,
            tc.tile_pool(name="dram", bufs=1, space="DRAM") as dram,
        ):
            # ---- constants ----
            w1 = cpool.tile([128, 3 * 24], BF16)
            d2 = cpool.tile([24, 3 * 128], BF16)
            w3 = cpool.tile([128, 3 * 36], BF16)
            d3a = cpool.tile([36, 3 * 12], BF16)
            d3b = cpool.tile([12, 3 * 128], BF16)
            b2 = cpool.tile([128, 1], F32)
            b3 = cpool.tile([128, 1], F32)
            gam = cpool.tile([128, 1], F32)
            bet = cpool.tile([128, 1], F32)
            for t_, e_ in ((w1, w1_e), (d2, d2_e), (w3, w3_e), (d3a, d3a_e),
                           (d3b, d3b_e), (b2, b2_e), (b3, b3_e),
                           (gam, gam_e), (bet, bet_e)):
                nc.sync.dma_start(t_[:], e_[:])

            X = xpool.tile([128, XLEN], BF16)
            P = xpool.tile([128, 4 * IN], BF16)          # P ring, packed planes
            TR = xpool.tile([12, 4 * PS], BF16)          # t ring
            sums = cpool.tile([128, T], F32)
            sumsqs = cpool.tile([128, T], F32)
            scr = cpool.tile([128, PS], BF16)            # sumsq scratch out

            def xbase(u):
                return PG + u * PS

            def xpl(u, off=0, ln=PS):
                return X[:, xbase(u) + off: xbase(u) + off + ln]

            def xint(u, rows=(0, 128)):
                base = xbase(u) + PG
                ap = X[rows[0]:rows[1], base: base + 56 * 58]
                return ap.rearrange("p (h w) -> p h w", h=56)[:, :, 0:56]

            def pflat(u, rows=(0, 128)):
                s = (u % 4) * IN
                return P[rows[0]:rows[1], s: s + IN]

            def pint(u, rows=(0, 128)):
                return pflat(u, rows).rearrange("p (h w) -> p h w", h=56)

            # ---- zero guards / borders (gpsimd memsets are cheap) ----
            nc.gpsimd.memset(X[:, 0:PG], 0.0)
            nc.gpsimd.memset(X[:, XLEN - PG: XLEN], 0.0)
            for u in range(T):
                b = xbase(u)
                nc.gpsimd.memset(X[:, b: b + 58], 0.0)                 # row 0
                nc.gpsimd.memset(X[:, b + 57 * 58: b + PS], 0.0)       # row 57
                ap = X[:, b: b + PS].rearrange("p (h w) -> p h w", h=58)
                nc.gpsimd.memset(ap[:, :, 0:1], 0.0)                   # col 0
                nc.gpsimd.memset(ap[:, :, 57:58], 0.0)                 # col 57

            # ---- input DMA (all 8 frames up front, SP queue) ----
            for u in range(T):
                nc.sync.dma_start(xint(u), x_e[u])

            # =============== phase A: conv2d attn + gating + stats ========
            attn_tiles = {}
            for u in range(T):
                s2 = spool.tile([24, ROWG + PS + ROWG], BF16,
                                name=f"s2_{u}", tag="s2")
                if u < 2:
                    nc.gpsimd.memset(s2[:, 0:ROWG], 0.0)
                    nc.gpsimd.memset(s2[:, ROWG + PS:], 0.0)
                for q in range(4):
                    ps = psum.tile([128, QUAR], F32, name=f"a1_{u}_{q}",
                                   tag="small")
                    for ck, ln in SUBS:
                        for dxi in range(3):
                            nc.tensor.matmul(
                                ps[0:24, ck:ck + ln],
                                w1[:, dxi * 24:(dxi + 1) * 24],
                                xpl(u, q * QUAR + ck + dxi - 1, ln),
                                start=(dxi == 0), stop=(dxi == 2))
                    nc.vector.tensor_copy(
                        s2[0:24, ROWG + q * QUAR: ROWG + (q + 1) * QUAR],
                        ps[0:24, :])
                at = apool.tile([128, PS], BF16, name=f"attn_{u}", tag="attn")
                attn_tiles[u] = at
                for q in range(4):
                    ps2 = psum.tile([128, QUAR], F32, name=f"a2_{u}_{q}",
                                    tag="big")
                    for ck, ln in SUBS:
                        for dyi in range(3):
                            nc.tensor.matmul(
                                ps2[:, ck:ck + ln],
                                d2[0:24, dyi * 128:(dyi + 1) * 128],
                                s2[0:24, ROWG + q * QUAR + ck
                                   + 58 * (dyi - 1): ROWG + q * QUAR + ck
                                   + 58 * (dyi - 1) + ln],
                                start=(dyi == 0), stop=(dyi == 2))
                    nc.scalar.activation(at[:, q * QUAR:(q + 1) * QUAR],
                                         ps2[:, :], AF.Sigmoid, bias=b2[:, 0:1])
                # x2 = max(attn, 0.5) * x  (in place), accumulate sums
                nc.vector.scalar_tensor_tensor(
                    out=xpl(u), in0=at[:, :], scalar=0.5, in1=xpl(u),
                    op0=ALU.max, op1=ALU.mult, accum_out=sums[:, u:u + 1])
                nc.scalar.activation(scr[:, :], xpl(u), AF.Square,
                                     accum_out=sumsqs[:, u:u + 1])

            # ---- BN stats ----
            red = cpool.tile([128, 2], F32)
            nc.vector.tensor_reduce(red[:, 0:1], sums[:], mybir.AxisListType.X,
                                    ALU.add)
            nc.vector.tensor_reduce(red[:, 1:2], sumsqs[:],
                                    mybir.AxisListType.X, ALU.add)
            if local_bn:
                ar = red
                ntot = NTOT_LOCAL
            else:
                cc_in = dram.tile([128, 2], F32)
                cc_out = dram.tile([128, 2], F32, addr_space="Shared")
                nc.sync.dma_start(cc_in[:], red[:])
                nc.gpsimd.collective_compute(
                    "AllReduce", ALU.add,
                    replica_groups=[list(range(N_CORES))],
                    ins=[cc_in[:].opt()], outs=[cc_out[:].opt()])
                ar = cpool.tile([128, 2], F32)
                nc.sync.dma_start(ar[:], cc_out[:])
                ntot = NTOT_GLOBAL

            st = cpool.tile([64, 8], F32)
            arl = cpool.tile([64, 2], F32)
            nc.vector.tensor_copy(arl[:, :], ar[64:128, :])
            nc.vector.tensor_tensor(st[:, 0:2], ar[0:64, :], arl[:, :], ALU.add)
            nc.vector.tensor_scalar_mul(st[:, 2:4], st[:, 0:2], 1.0 / ntot)
            nc.vector.tensor_tensor(st[:, 4:5], st[:, 2:3], st[:, 2:3], ALU.mult)
            nc.vector.tensor_tensor(st[:, 4:5], st[:, 3:4], st[:, 4:5],
                                    ALU.subtract)
            nc.vector.tensor_scalar_add(st[:, 4:5], st[:, 4:5], EPS)
            # rsqrt: s = sqrt(v); r = 1/s; Newton: r = r*(2 - s*r)
            nc.scalar.activation(st[:, 5:6], st[:, 4:5], AF.Sqrt)
            nc.vector.reciprocal(st[:, 6:7], st[:, 5:6])
            nc.vector.tensor_tensor(st[:, 7:8], st[:, 5:6], st[:, 6:7], ALU.mult)
            nc.vector.tensor_scalar(st[:, 7:8], st[:, 7:8], -1.0, 2.0,
                                    ALU.mult, ALU.add)
            nc.vector.tensor_tensor(st[:, 6:7], st[:, 6:7], st[:, 7:8], ALU.mult)
            scb = cpool.tile([128, 2], F32)
            nc.vector.tensor_tensor(scb[0:64, 0:1], gam[0:64, :], st[:, 6:7],
                                    ALU.mult)
            nc.vector.tensor_tensor(st[:, 7:8], st[:, 2:3], scb[0:64, 0:1],
                                    ALU.mult)
            nc.vector.tensor_tensor(scb[0:64, 1:2], bet[0:64, :], st[:, 7:8],
                                    ALU.subtract)
            nc.vector.tensor_copy(scb[64:128, :], scb[0:64, :])

            # =============== phase B: bnrelu + conv3d + combine ===========
            def bnrelu(u):
                bt = spool.tile([128, PG + PS + PG], BF16,
                                name=f"bnr_{u}", tag="bnr")
                if u < 2:
                    nc.gpsimd.memset(bt[:, 0:PG + 58], 0.0)
                    nc.gpsimd.memset(bt[:, PG + 57 * 58:], 0.0)
                    ap = bt[:, PG: PG + PS].rearrange("p (h w) -> p h w", h=58)
                    nc.gpsimd.memset(ap[:, :, 0:1], 0.0)
                    nc.gpsimd.memset(ap[:, :, 57:58], 0.0)
                dst = bt[:, PG + PG: PG + PG + 56 * 58].rearrange(
                    "p (h w) -> p h w", h=56)[:, :, 0:56]
                nc.scalar.activation(dst, xint(u), AF.Relu,
                                     bias=scb[:, 1:2], scale=scb[:, 0:1])
                return bt

            def conv3d_front(u, bt):
                # stage dx -> s3 [36, plane], stage dy -> t-ring [12, plane]
                s3 = spool.tile([36, ROWG + PS + ROWG], BF16,
                                name=f"s3_{u}", tag="s3")
                if u < 2:
                    nc.gpsimd.memset(s3[:, 0:ROWG], 0.0)
                    nc.gpsimd.memset(s3[:, ROWG + PS:], 0.0)
                for q in range(4):
                    ps = psum.tile([128, QUAR], F32, name=f"c1_{u}_{q}",
                                   tag="small")
                    for ck, ln in SUBS:
                        for dxi in range(3):
                            nc.tensor.matmul(
                                ps[0:36, ck:ck + ln],
                                w3[:, dxi * 36:(dxi + 1) * 36],
                                bt[:, PG + q * QUAR + ck + dxi - 1:
                                   PG + q * QUAR + ck + dxi - 1 + ln],
                                start=(dxi == 0), stop=(dxi == 2))
                    nc.scalar.activation(
                        s3[0:36, ROWG + q * QUAR: ROWG + (q + 1) * QUAR],
                        ps[0:36, :], AF.Copy)
                for q in range(4):
                    ps = psum.tile([128, QUAR], F32, name=f"c2_{u}_{q}",
                                   tag="small")
                    for ck, ln in SUBS:
                        for dyi in range(3):
                            nc.tensor.matmul(
                                ps[0:12, ck:ck + ln],
                                d3a[0:36, dyi * 12:(dyi + 1) * 12],
                                s3[0:36, ROWG + q * QUAR + ck
                                   + 58 * (dyi - 1): ROWG + q * QUAR + ck
                                   + 58 * (dyi - 1) + ln],
                                start=(dyi == 0), stop=(dyi == 2))
                    nc.vector.tensor_copy(
                        TR[0:12, (u % 4) * PS + q * QUAR:
                           (u % 4) * PS + (q + 1) * QUAR], ps[0:12, :])

            def conv3d_back(v):
                # stage dt from t-ring -> tanh gate -> P[v]
                dts = [dti for dti in range(3) if 0 <= v + dti - 1 < T]
                gt = gpool.tile([128, PS], BF16, name=f"gate_{v}", tag="gate")
                for q in range(4):
                    ps = psum.tile([128, QUAR], F32, name=f"c3_{v}_{q}",
                                   tag="big")
                    for ck, ln in SUBS:
                        for i, dti in enumerate(dts):
                            s = ((v + dti - 1) % 4) * PS
                            nc.tensor.matmul(
                                ps[:, ck:ck + ln],
                                d3b[0:12, dti * 128:(dti + 1) * 128],
                                TR[0:12, s + q * QUAR + ck:
                                   s + q * QUAR + ck + ln],
                                start=(i == 0), stop=(i == len(dts) - 1))
                    nc.scalar.activation(gt[:, q * QUAR:(q + 1) * QUAR],
                                         ps[:, :], AF.Tanh, bias=b3[:, 0:1])
                gint = gt[:, PG: PG + 56 * 58].rearrange(
                    "p (h w) -> p h w", h=56)[:, :, 0:56]
                nc.vector.tensor_tensor(pint(v), gint, xint(v), ALU.mult)

            def emit(w):
                yt = ypool.tile([128, IN], BF16, name=f"y_{w}", tag="y")
                yv = yt[:].rearrange("p (h w) -> p h w", h=56)
                nc.vector.tensor_tensor(yv, xint(w), pint(w), ALU.subtract)
                # g1 (h=0: parts 0:32, 64:96) += P[w+1]; g2 += P[w-1]
                if w + 1 < T:
                    nc.vector.tensor_tensor(yt[0:32, :], yt[0:32, :],
                                            pflat(w + 1, (0, 32)), ALU.add)
                    nc.gpsimd.tensor_tensor(yt[64:96, :], yt[64:96, :],
                                            pflat(w + 1, (64, 96)), ALU.add)
                if w - 1 >= 0:
                    nc.vector.tensor_tensor(yt[32:64, :], yt[32:64, :],
                                            pflat(w - 1, (32, 64)), ALU.add)
                    nc.gpsimd.tensor_tensor(yt[96:128, :], yt[96:128, :],
                                            pflat(w - 1, (96, 128)), ALU.add)
                nc.sync.dma_start(out_e[w], yt[:])

            for u in range(T):
                bt = bnrelu(u)
                conv3d_front(u, bt)
                if u >= 1:
                    conv3d_back(u - 1)
                if u >= 2:
                    emit(u - 2)
            conv3d_back(T - 1)
            emit(T - 2)
            emit(T - 1)

    return nc


CHMAP = np.array([32 * (q // 32) + 16 * ((q % 32) % 2) + (q % 32) // 2
                  for q in range(64)])


def _host_prep(attn_w, attn_b, bn_gamma, bn_beta, conv3d_w, conv3d_b):
    bf = ml_dtypes.bfloat16
    ch = np.concatenate([CHMAP, CHMAP])          # ch(p), p = 64*clip + q
    clip = np.arange(128) // 64
    hh = (np.arange(128) % 64) // 32             # group/half of partition

    W1 = np.zeros((128, 3 * 24), np.float32)
    for dxi in range(3):
        for dyi in range(3):
            for head in range(4):
                for cl in range(2):
                    col = dxi * 24 + dyi * 8 + head * 2 + cl
                    m = clip == cl
                    W1[m, col] = attn_w[head, ch[m], dyi, dxi]
    D2 = np.zeros((24, 3 * 128), np.float32)
    for dyi in range(3):
        for p in range(128):
            head = ch[p] // 16
            D2[dyi * 8 + head * 2 + clip[p], dyi * 128 + p] = 1.0
    W3 = np.zeros((128, 3 * 36), np.float32)
    for dxi in range(3):
        for dti in range(3):
            for dyi in range(3):
                for cl in range(2):
                    for g in range(2):
                        col = dxi * 36 + dti * 12 + dyi * 4 + g * 2 + cl
                        m = (clip == cl) & (hh == g)
                        W3[m, col] = conv3d_w[g, ch[m] % 32, dti, dyi, dxi]
    D3A = np.zeros((36, 3 * 12), np.float32)
    for dyi in range(3):
        for dti in range(3):
            for g in range(2):
                for cl in range(2):
                    D3A[dti * 12 + dyi * 4 + g * 2 + cl,
                        dyi * 12 + dti * 4 + g * 2 + cl] = 1.0
    D3B = np.zeros((12, 3 * 128), np.float32)
    for dti in range(3):
        for p in range(128):
            D3B[dti * 4 + hh[p] * 2 + clip[p], dti * 128 + p] = 1.0
    b2r = attn_b[ch // 16].astype(np.float32).reshape(128, 1)
    b3r = conv3d_b[hh].astype(np.float32).reshape(128, 1)
    gr = bn_gamma[ch].astype(np.float32).reshape(128, 1)
    br = bn_beta[ch].astype(np.float32).reshape(128, 1)
    return (W1.astype(bf), D2.astype(bf), W3.astype(bf), D3A.astype(bf),
            D3B.astype(bf), b2r, b3r, gr, br)


def get_nc():
    if "nc" not in _CACHE:
        nc = build_nc()
        _legalize_waits(nc)
        _CACHE["nc"] = nc
    return _CACHE["nc"]


def make_in_maps(x, attn_w, attn_b, bn_gamma, bn_beta, conv3d_w, conv3d_b):
    W1, D2, W3, D3A, D3B, b2r, b3r, gr, br = _host_prep(
        np.asarray(attn_w), np.asarray(attn_b), np.asarray(bn_gamma),
        np.asarray(bn_beta), np.asarray(conv3d_w), np.asarray(conv3d_b))
    x = np.asarray(x)
    bf = ml_dtypes.bfloat16
    in_maps = []
    for k in range(N_CORES):
        xk = x[16 * k:16 * (k + 1)].reshape(2, T, 64, 56, 56)
        xp = np.ascontiguousarray(
            xk[:, :, CHMAP].transpose(1, 0, 2, 3, 4).reshape(
                T, 128, 56, 56)).astype(bf)
        in_maps.append({
            "x": xp, "w1": W1, "d2": D2, "w3": W3, "d3a": D3A, "d3b": D3B,
            "b2": b2r, "b3": b3r, "gam": gr, "bet": br,
        })
    return in_maps


def kernel(x, attn_w, attn_b, bn_gamma, bn_beta, conv3d_w, conv3d_b):
    nc = get_nc()
    in_maps = make_in_maps(x, attn_w, attn_b, bn_gamma, bn_beta,
                           conv3d_w, conv3d_b)
    res = run_bass_kernel_spmd(nc, in_maps, core_ids=list(range(N_CORES)))
    parts = []
    for k in range(N_CORES):
        r = np.asarray(res.results[k]["out"]).astype(np.float32)
        parts.append(r.reshape(T, 2, 64, 56, 56).transpose(1, 0, 2, 3, 4)
                      .reshape(16, 64, 56, 56))
    return np.concatenate(parts, 0)


# revision 12
# speedup vs baseline: 4.6998x; 4.6998x over previous
"""AGSM Trainium2 kernel: attention-gated temporal shift module on 8 NeuronCores.

Sharding: data-parallel over clips. B=16 clips; core k handles clips (2k, 2k+1).
BN batch stats via tiny AllReduce (or local-BN per shard behind a flag).

Per-core layout (per frame u, 128 partitions): p = 64*clip + q where partition
q holds input channel chmap[q] = 32*(q//32) + 16*((q%32)%2) + (q%32)//2, chosen
so that partition order == final interleaved OUTPUT channel order (stores are
then a single contiguous 128-partition DMA per frame).

Convs are factorized into shift-sum matmul stages (all free-axis tap offsets):
  conv2d: [dx: 3 streams, K=128 -> 24 (dy,head,clip)] -> [dy: 3 streams,
          24 -> 128 replicated attn] -> sigmoid
  conv3d: [dx: 3 -> 36 (dt,dy,g,clip)] -> [dy: 3 -> 12 (dt,g,clip)] ->
          [dt: 3 over t-ring -> 128 replicated gate] -> tanh
This cuts PE column traffic ~4.8x vs direct per-tap matmuls.

Spatial planes padded to 58x58 with zero borders so taps are free-axis offsets.
x is host-pre-cast to bf16 + channel-permuted; output leaves as bf16 [8,128,...]
and is cast/unpermuted on host.

build_nc(repeat=R) replays the whole pass R times in one NEFF for differential
timing (tile tags make all per-rep tiles reuse the same buffers).
"""
import numpy as np
import ml_dtypes

import concourse.bass as bass
import concourse.tile as tile
from concourse import mybir
from concourse.bass_utils import run_bass_kernel_spmd

N_CORES = 8
T = 8
PS = 3364            # padded plane 58*58
IN = 3136            # interior 56*56
ROWG = 58            # guard for row shifts (s2/s3 tiles)
PG = 59              # guard for X/bnr plane tiles
QUAR = 841
EPS = 1e-5
LOCAL_BN = False
NTOT_GLOBAL = 16 * T * IN
NTOT_LOCAL = 2 * T * IN
F32 = mybir.dt.float32
BF16 = mybir.dt.bfloat16
AF = mybir.ActivationFunctionType
ALU = mybir.AluOpType

_CACHE = {}

SUBS = ((0, 512), (512, 329))    # psum bank-safe sub-chunks of a quarter


def _legalize_waits(nc):
    """This walrus accepts <=1 sync wait per instruction (2 for EventSemaphore).
    Hoist excess waits onto fresh same-engine NoOps inserted just before."""
    n = [0]
    for f in nc.m.functions:
        for bb in f.blocks:
            insts = bb.instructions  # live list
            i = 0
            while i < len(insts):
                inst = insts[i]
                si = inst.sync_info
                cap = 2 if type(inst).__name__ == "InstEventSemaphore" else 1
                if si is not None and len(si.on_wait) > cap:
                    waits = list(si.on_wait)
                    si.on_wait = waits[-cap:]
                    inst.sync_info = si
                    for w in waits[:-cap]:
                        n[0] += 1
                        nop = mybir.InstNoOp(
                            name=f"waitfix-{n[0]}", engine=inst.engine,
                            bass_nofuse=True,
                            sync_info=mybir.SyncInfo(on_wait=[w], on_update=[]))
                        nc.register_instruction(nop, overwrite=True)
                        insts.insert(i, nop)
                        i += 1
                i += 1


def build_nc(local_bn=LOCAL_BN, repeat=1):
    nc = bass.Bass(num_devices=N_CORES)
    x_e = nc.declare_dram_parameter("x", [T, 128, 56, 56], BF16, isOutput=False)
    w1_e = nc.declare_dram_parameter("w1", [128, 3 * 24], BF16, isOutput=False)
    d2_e = nc.declare_dram_parameter("d2", [24, 3 * 128], BF16, isOutput=False)
    w3_e = nc.declare_dram_parameter("w3", [128, 3 * 36], BF16, isOutput=False)
    d3a_e = nc.declare_dram_parameter("d3a", [36, 3 * 12], BF16, isOutput=False)
    d3b_e = nc.declare_dram_parameter("d3b", [12, 3 * 128], BF16, isOutput=False)
    b2_e = nc.declare_dram_parameter("b2", [128, 1], F32, isOutput=False)
    b3_e = nc.declare_dram_parameter("b3", [128, 1], F32, isOutput=False)
    gam_e = nc.declare_dram_parameter("gam", [128, 1], F32, isOutput=False)
    bet_e = nc.declare_dram_parameter("bet", [128, 1], F32, isOutput=False)
    out_e = nc.declare_dram_parameter("out", [T, 128, 56, 56], BF16, isOutput=True)

    XLEN = PG + T * PS + PG

    with tile.TileContext(nc) as tc:
        with (
            tc.tile_pool(name="const", bufs=1) as cpool,
            tc.tile_pool(name="xbuf", bufs=1) as xpool,
            tc.tile_pool(name="stg", bufs=2) as spool,      # s2/s3/bnr staging
            tc.tile_pool(name="attn", bufs=2) as apool,
            tc.tile_pool(name="gate", bufs=2) as gpool,
            tc.tile_pool(name="ybuf", bufs=2) as ypool,
            tc.tile_pool(name="psum", bufs=2, space=bass.MemorySpace.PSUM) as psum,
            tc.tile_pool(name="dram", bufs=1, space="DRAM") as dram,
        ):
            # ---- constants ----
            w1 = cpool.tile([128, 3 * 24], BF16)
            d2 = cpool.tile([24, 3 * 128], BF16)
            w3 = cpool.tile([128, 3 * 36], BF16)
            d3a = cpool.tile([36, 3 * 12], BF16)
            d3b = cpool.tile([12, 3 * 128], BF16)
            b2 = cpool.tile([128, 1], F32)
            b3 = cpool.tile([128, 1], F32)
            gam = cpool.tile([128, 1], F32)
            bet = cpool.tile([128, 1], F32)
            for t_, e_ in ((w1, w1_e), (d2, d2_e), (w3, w3_e), (d3a, d3a_e),
                           (d3b, d3b_e), (b2, b2_e), (b3, b3_e),
                           (gam, gam_e), (bet, bet_e)):
                nc.sync.dma_start(t_[:], e_[:])

            X = xpool.tile([128, XLEN], BF16)
            P = xpool.tile([128, 4 * IN], BF16)          # P ring, packed planes
            TR = xpool.tile([12, 4 * PS], BF16)          # t ring
            sums = cpool.tile([128, T], F32)
            sumsqs = cpool.tile([128, T], F32)
            scr = cpool.tile([128, PS], BF16)            # sumsq scratch out
            red = cpool.tile([128, 2], F32)
            st = cpool.tile([64, 8], F32)
            arl = cpool.tile([64, 2], F32)
            scb = cpool.tile([128, 2], F32)
            if local_bn:
                arg = red
            else:
                arg = cpool.tile([128, 2], F32)

            def xbase(u):
                return PG + u * PS

            def xpl(u, off=0, ln=PS):
                return X[:, xbase(u) + off: xbase(u) + off + ln]

            def xint(u, rows=(0, 128)):
                base = xbase(u) + PG
                ap = X[rows[0]:rows[1], base: base + 56 * 58]
                return ap.rearrange("p (h w) -> p h w", h=56)[:, :, 0:56]

            def pflat(u, rows=(0, 128)):
                s = (u % 4) * IN
                return P[rows[0]:rows[1], s: s + IN]

            def pint(u, rows=(0, 128)):
                return pflat(u, rows).rearrange("p (h w) -> p h w", h=56)

            # ---- zero guards / borders (gpsimd memsets are cheap) ----
            nc.gpsimd.memset(X[:, 0:PG], 0.0)
            nc.gpsimd.memset(X[:, XLEN - PG: XLEN], 0.0)
            for u in range(T):
                b = xbase(u)
                nc.gpsimd.memset(X[:, b: b + 58], 0.0)                 # row 0
                nc.gpsimd.memset(X[:, b + 57 * 58: b + PS], 0.0)       # row 57
                ap = X[:, b: b + PS].rearrange("p (h w) -> p h w", h=58)
                nc.gpsimd.memset(ap[:, :, 0:1], 0.0)                   # col 0
                nc.gpsimd.memset(ap[:, :, 57:58], 0.0)                 # col 57

            for rep in range(repeat):
                # ---- input DMA (all 8 frames up front, SP queue) ----
                for u in range(T):
                    nc.sync.dma_start(xint(u), x_e[u])

                # ======= phase A: conv2d attn + gating + stats =======
                for u in range(T):
                    s2 = spool.tile([24, ROWG + PS + ROWG], BF16,
                                    name=f"s2_{rep}_{u}", tag="s2")
                    if rep == 0 and u < 2:
                        nc.gpsimd.memset(s2[:, 0:ROWG], 0.0)
                        nc.gpsimd.memset(s2[:, ROWG + PS:], 0.0)
                    for q in range(4):
                        ps = psum.tile([128, QUAR], F32,
                                       name=f"a1_{rep}_{u}_{q}", tag="small")
                        for ck, ln in SUBS:
                            for dxi in range(3):
                                nc.tensor.matmul(
                                    ps[0:24, ck:ck + ln],
                                    w1[:, dxi * 24:(dxi + 1) * 24],
                                    xpl(u, q * QUAR + ck + dxi - 1, ln),
                                    start=(dxi == 0), stop=(dxi == 2))
                        nc.vector.tensor_copy(
                            s2[0:24, ROWG + q * QUAR: ROWG + (q + 1) * QUAR],
                            ps[0:24, :])
                    at = apool.tile([128, PS], BF16, name=f"attn_{rep}_{u}",
                                    tag="attn")
                    for q in range(4):
                        ps2 = psum.tile([128, QUAR], F32,
                                        name=f"a2_{rep}_{u}_{q}", tag="big")
                        for ck, ln in SUBS:
                            for dyi in range(3):
                                o = ROWG + q * QUAR + ck + 58 * (dyi - 1)
                                nc.tensor.matmul(
                                    ps2[:, ck:ck + ln],
                                    d2[0:24, dyi * 128:(dyi + 1) * 128],
                                    s2[0:24, o: o + ln],
                                    start=(dyi == 0), stop=(dyi == 2))
                        nc.scalar.activation(at[:, q * QUAR:(q + 1) * QUAR],
                                             ps2[:, :], AF.Sigmoid,
                                             bias=b2[:, 0:1])
                    # x2 = max(attn, 0.5) * x  (in place), accumulate sums
                    nc.vector.scalar_tensor_tensor(
                        out=xpl(u), in0=at[:, :], scalar=0.5, in1=xpl(u),
                        op0=ALU.max, op1=ALU.mult, accum_out=sums[:, u:u + 1])
                    nc.scalar.activation(scr[:, :], xpl(u), AF.Square,
                                         accum_out=sumsqs[:, u:u + 1])

                # ---- BN stats ----
                nc.vector.tensor_reduce(red[:, 0:1], sums[:],
                                        mybir.AxisListType.X, ALU.add)
                nc.vector.tensor_reduce(red[:, 1:2], sumsqs[:],
                                        mybir.AxisListType.X, ALU.add)
                if local_bn:
                    ntot = NTOT_LOCAL
                else:
                    cc_in = dram.tile([128, 2], F32, name=f"cc_in_{rep}")
                    cc_out = dram.tile([128, 2], F32, addr_space="Shared",
                                       name=f"cc_out_{rep}")
                    nc.sync.dma_start(cc_in[:], red[:])
                    nc.gpsimd.collective_compute(
                        "AllReduce", ALU.add,
                        replica_groups=[list(range(N_CORES))],
                        ins=[cc_in[:].opt()], outs=[cc_out[:].opt()])
                    nc.sync.dma_start(arg[:], cc_out[:])
                    ntot = NTOT_GLOBAL

                nc.vector.tensor_copy(arl[:, :], arg[64:128, :])
                nc.vector.tensor_tensor(st[:, 0:2], arg[0:64, :], arl[:, :],
                                        ALU.add)
                nc.vector.tensor_scalar_mul(st[:, 2:4], st[:, 0:2], 1.0 / ntot)
                nc.vector.tensor_tensor(st[:, 4:5], st[:, 2:3], st[:, 2:3],
                                        ALU.mult)
                nc.vector.tensor_tensor(st[:, 4:5], st[:, 3:4], st[:, 4:5],
                                        ALU.subtract)
                nc.vector.tensor_scalar_add(st[:, 4:5], st[:, 4:5], EPS)
                # rsqrt: s = sqrt(v); r = 1/s; Newton: r = r*(2 - s*r)
                nc.scalar.activation(st[:, 5:6], st[:, 4:5], AF.Sqrt)
                nc.vector.reciprocal(st[:, 6:7], st[:, 5:6])
                nc.vector.tensor_tensor(st[:, 7:8], st[:, 5:6], st[:, 6:7],
                                        ALU.mult)
                nc.vector.tensor_scalar(st[:, 7:8], st[:, 7:8], -1.0, 2.0,
                                        ALU.mult, ALU.add)
                nc.vector.tensor_tensor(st[:, 6:7], st[:, 6:7], st[:, 7:8],
                                        ALU.mult)
                nc.vector.tensor_tensor(scb[0:64, 0:1], gam[0:64, :],
                                        st[:, 6:7], ALU.mult)
                nc.vector.tensor_tensor(st[:, 7:8], st[:, 2:3], scb[0:64, 0:1],
                                        ALU.mult)
                nc.vector.tensor_tensor(scb[0:64, 1:2], bet[0:64, :],
                                        st[:, 7:8], ALU.subtract)
                nc.vector.tensor_copy(scb[64:128, :], scb[0:64, :])

                # ======= phase B: bnrelu + conv3d + combine =======
                def bnrelu(u):
                    bt = spool.tile([128, PG + PS + PG], BF16,
                                    name=f"bnr_{rep}_{u}", tag="bnr")
                    if rep == 0 and u < 2:
                        nc.gpsimd.memset(bt[:, 0:PG + 58], 0.0)
                        nc.gpsimd.memset(bt[:, PG + 57 * 58:], 0.0)
                        ap = bt[:, PG: PG + PS].rearrange(
                            "p (h w) -> p h w", h=58)
                        nc.gpsimd.memset(ap[:, :, 0:1], 0.0)
                        nc.gpsimd.memset(ap[:, :, 57:58], 0.0)
                    dst = bt[:, PG + PG: PG + PG + 56 * 58].rearrange(
                        "p (h w) -> p h w", h=56)[:, :, 0:56]
                    nc.scalar.activation(dst, xint(u), AF.Relu,
                                         bias=scb[:, 1:2], scale=scb[:, 0:1])
                    return bt

                def conv3d_front(u, bt):
                    s3 = spool.tile([36, ROWG + PS + ROWG], BF16,
                                    name=f"s3_{rep}_{u}", tag="s3")
                    if rep == 0 and u < 2:
                        nc.gpsimd.memset(s3[:, 0:ROWG], 0.0)
                        nc.gpsimd.memset(s3[:, ROWG + PS:], 0.0)
                    for q in range(4):
                        ps = psum.tile([128, QUAR], F32,
                                       name=f"c1_{rep}_{u}_{q}", tag="small")
                        for ck, ln in SUBS:
                            for dxi in range(3):
                                o = PG + q * QUAR + ck + dxi - 1
                                nc.tensor.matmul(
                                    ps[0:36, ck:ck + ln],
                                    w3[:, dxi * 36:(dxi + 1) * 36],
                                    bt[:, o: o + ln],
                                    start=(dxi == 0), stop=(dxi == 2))
                        nc.scalar.activation(
                            s3[0:36, ROWG + q * QUAR: ROWG + (q + 1) * QUAR],
                            ps[0:36, :], AF.Copy)
                    for q in range(4):
                        ps = psum.tile([128, QUAR], F32,
                                       name=f"c2_{rep}_{u}_{q}", tag="small")
                        for ck, ln in SUBS:
                            for dyi in range(3):
                                o = ROWG + q * QUAR + ck + 58 * (dyi - 1)
                                nc.tensor.matmul(
                                    ps[0:12, ck:ck + ln],
                                    d3a[0:36, dyi * 12:(dyi + 1) * 12],
                                    s3[0:36, o: o + ln],
                                    start=(dyi == 0), stop=(dyi == 2))
                        nc.vector.tensor_copy(
                            TR[0:12, (u % 4) * PS + q * QUAR:
                               (u % 4) * PS + (q + 1) * QUAR], ps[0:12, :])

                def conv3d_back(v):
                    dts = [dti for dti in range(3) if 0 <= v + dti - 1 < T]
                    gt = gpool.tile([128, PS], BF16, name=f"gate_{rep}_{v}",
                                    tag="gate")
                    for q in range(4):
                        ps = psum.tile([128, QUAR], F32,
                                       name=f"c3_{rep}_{v}_{q}", tag="big")
                        for ck, ln in SUBS:
                            for i, dti in enumerate(dts):
                                s = ((v + dti - 1) % 4) * PS
                                nc.tensor.matmul(
                                    ps[:, ck:ck + ln],
                                    d3b[0:12, dti * 128:(dti + 1) * 128],
                                    TR[0:12, s + q * QUAR + ck:
                                       s + q * QUAR + ck + ln],
                                    start=(i == 0), stop=(i == len(dts) - 1))
                        nc.scalar.activation(gt[:, q * QUAR:(q + 1) * QUAR],
                                             ps[:, :], AF.Tanh, bias=b3[:, 0:1])
                    gint = gt[:, PG: PG + 56 * 58].rearrange(
                        "p (h w) -> p h w", h=56)[:, :, 0:56]
                    nc.vector.tensor_tensor(pint(v), gint, xint(v), ALU.mult)

                def emit(w):
                    yt = ypool.tile([128, IN], BF16, name=f"y_{rep}_{w}",
                                    tag="y")
                    yv = yt[:].rearrange("p (h w) -> p h w", h=56)
                    nc.vector.tensor_tensor(yv, xint(w), pint(w), ALU.subtract)
                    # g1 (h=0: parts 0:32, 64:96) += P[w+1]; g2 += P[w-1]
                    if w + 1 < T:
                        nc.vector.tensor_tensor(yt[0:32, :], yt[0:32, :],
                                                pflat(w + 1, (0, 32)), ALU.add)
                        nc.gpsimd.tensor_tensor(yt[64:96, :], yt[64:96, :],
                                                pflat(w + 1, (64, 96)),
                                                ALU.add)
                    if w - 1 >= 0:
                        nc.vector.tensor_tensor(yt[32:64, :], yt[32:64, :],
                                                pflat(w - 1, (32, 64)),
                                                ALU.add)
                        nc.gpsimd.tensor_tensor(yt[96:128, :], yt[96:128, :],
                                                pflat(w - 1, (96, 128)),
                                                ALU.add)
                    nc.sync.dma_start(out_e[w], yt[:])

                for u in range(T):
                    bt = bnrelu(u)
                    conv3d_front(u, bt)
                    if u >= 1:
                        conv3d_back(u - 1)
                    if u >= 2:
                        emit(u - 2)
                conv3d_back(T - 1)
                emit(T - 2)
                emit(T - 1)

    return nc


CHMAP = np.array([32 * (q // 32) + 16 * ((q % 32) % 2) + (q % 32) // 2
                  for q in range(64)])


def _host_prep(attn_w, attn_b, bn_gamma, bn_beta, conv3d_w, conv3d_b):
    bf = ml_dtypes.bfloat16
    ch = np.concatenate([CHMAP, CHMAP])          # ch(p), p = 64*clip + q
    clip = np.arange(128) // 64
    hh = (np.arange(128) % 64) // 32             # group/half of partition

    W1 = np.zeros((128, 3 * 24), np.float32)
    for dxi in range(3):
        for dyi in range(3):
            for head in range(4):
                for cl in range(2):
                    col = dxi * 24 + dyi * 8 + head * 2 + cl
                    m = clip == cl
                    W1[m, col] = attn_w[head, ch[m], dyi, dxi]
    D2 = np.zeros((24, 3 * 128), np.float32)
    for dyi in range(3):
        for p in range(128):
            head = ch[p] // 16
            D2[dyi * 8 + head * 2 + clip[p], dyi * 128 + p] = 1.0
    W3 = np.zeros((128, 3 * 36), np.float32)
    for dxi in range(3):
        for dti in range(3):
            for dyi in range(3):
                for cl in range(2):
                    for g in range(2):
                        col = dxi * 36 + dti * 12 + dyi * 4 + g * 2 + cl
                        m = (clip == cl) & (hh == g)
                        W3[m, col] = conv3d_w[g, ch[m] % 32, dti, dyi, dxi]
    D3A = np.zeros((36, 3 * 12), np.float32)
    for dyi in range(3):
        for dti in range(3):
            for g in range(2):
                for cl in range(2):
                    D3A[dti * 12 + dyi * 4 + g * 2 + cl,
                        dyi * 12 + dti * 4 + g * 2 + cl] = 1.0
    D3B = np.zeros((12, 3 * 128), np.float32)
    for dti in range(3):
        for p in range(128):
            D3B[dti * 4 + hh[p] * 2 + clip[p], dti * 128 + p] = 1.0
    b2r = attn_b[ch // 16].astype(np.float32).reshape(128, 1)
    b3r = conv3d_b[hh].astype(np.float32).reshape(128, 1)
    gr = bn_gamma[ch].astype(np.float32).reshape(128, 1)
    br = bn_beta[ch].astype(np.float32).reshape(128, 1)
    return (W1.astype(bf), D2.astype(bf), W3.astype(bf), D3A.astype(bf),
            D3B.astype(bf), b2r, b3r, gr, br)


def get_nc():
    if "nc" not in _CACHE:
        nc = build_nc()
        _legalize_waits(nc)
        _CACHE["nc"] = nc
    return _CACHE["nc"]


def make_in_maps(x, attn_w, attn_b, bn_gamma, bn_beta, conv3d_w, conv3d_b):
    W1, D2, W3, D3A, D3B, b2r, b3r, gr, br = _host_prep(
        np.asarray(attn_w), np.asarray(attn_b), np.asarray(bn_gamma),
        np.asarray(bn_beta), np.asarray(conv3d_w), np.asarray(conv3d_b))
    x = np.asarray(x)
    bf = ml_dtypes.bfloat16
    in_maps = []
    for k in range(N_CORES):
        xk = x[16 * k:16 * (k + 1)].reshape(2, T, 64, 56, 56)
        xp = np.ascontiguousarray(
            xk[:, :, CHMAP].transpose(1, 0, 2, 3, 4).reshape(
                T, 128, 56, 56)).astype(bf)
        in_maps.append({
            "x": xp, "w1": W1, "d2": D2, "w3": W3, "d3a": D3A, "d3b": D3B,
            "b2": b2r, "b3": b3r, "gam": gr, "bet": br,
        })
    return in_maps


def kernel(x, attn_w, attn_b, bn_gamma, bn_beta, conv3d_w, conv3d_b):
    nc = get_nc()
    in_maps = make_in_maps(x, attn_w, attn_b, bn_gamma, bn_beta,
                           conv3d_w, conv3d_b)
    res = run_bass_kernel_spmd(nc, in_maps, core_ids=list(range(N_CORES)))
    parts = []
    for k in range(N_CORES):
        r = np.asarray(res.results[k]["out"]).astype(np.float32)
        parts.append(r.reshape(T, 2, 64, 56, 56).transpose(1, 0, 2, 3, 4)
                      .reshape(16, 64, 56, 56))
    return np.concatenate(parts, 0)


# revision 13
# speedup vs baseline: 5.7382x; 1.2209x over previous
"""AGSM Trainium2 kernel: attention-gated temporal shift module on 8 NeuronCores.

Sharding: data-parallel over clips. B=16 clips; core k handles clips (2k, 2k+1).
BN batch stats via tiny AllReduce (or local-BN per shard behind a flag).

Per-core layout (per frame u, 128 partitions): p = 64*clip + q where partition
q holds input channel chmap[q] = 32*(q//32) + 16*((q%32)%2) + (q%32)//2, chosen
so that partition order == final interleaved OUTPUT channel order (stores are
then a single contiguous 128-partition DMA per frame).

Convs are factorized into shift-sum matmul stages (all free-axis tap offsets):
  conv2d: [dx: 3 streams, K=128 -> 24 (dy,head,clip)] -> [dy: 3 streams,
          24 -> 128 replicated attn] -> sigmoid
  conv3d: [dx: 3 -> 36 (dt,dy,g,clip)] -> [dy: 3 -> 12 (dt,g,clip)] ->
          [dt: 3 over t-ring -> 128 replicated gate] -> tanh
This cuts PE column traffic ~4.8x vs direct per-tap matmuls.

Spatial planes padded to 58x58 with zero borders so taps are free-axis offsets.
x is host-pre-cast to bf16 + channel-permuted; output leaves as bf16 [8,128,...]
and is cast/unpermuted on host.

build_nc(repeat=R) replays the whole pass R times in one NEFF for differential
timing (tile tags make all per-rep tiles reuse the same buffers).
"""
import numpy as np
import ml_dtypes

import concourse.bass as bass
import concourse.tile as tile
from concourse import mybir
from concourse.bass_utils import run_bass_kernel_spmd

N_CORES = 8
T = 8
PS = 3364            # padded plane 58*58
IN = 3136            # interior 56*56
ROWG = 58            # guard for row shifts (s2/s3 tiles)
PG = 59              # guard for X/bnr plane tiles
QUAR = 841
EPS = 1e-5
LOCAL_BN = True
NTOT_GLOBAL = 16 * T * IN
NTOT_LOCAL = 2 * T * IN
F32 = mybir.dt.float32
BF16 = mybir.dt.bfloat16
AF = mybir.ActivationFunctionType
ALU = mybir.AluOpType

_CACHE = {}

SUBS = ((0, 512), (512, 329))    # psum bank-safe sub-chunks of a quarter


def _legalize_waits(nc):
    """This walrus accepts <=1 sync wait per instruction (2 for EventSemaphore).
    Hoist excess waits onto fresh same-engine NoOps inserted just before."""
    n = [0]
    for f in nc.m.functions:
        for bb in f.blocks:
            insts = bb.instructions  # live list
            i = 0
            while i < len(insts):
                inst = insts[i]
                si = inst.sync_info
                cap = 2 if type(inst).__name__ == "InstEventSemaphore" else 1
                if si is not None and len(si.on_wait) > cap:
                    waits = list(si.on_wait)
                    si.on_wait = waits[-cap:]
                    inst.sync_info = si
                    for w in waits[:-cap]:
                        n[0] += 1
                        nop = mybir.InstNoOp(
                            name=f"waitfix-{n[0]}", engine=inst.engine,
                            bass_nofuse=True,
                            sync_info=mybir.SyncInfo(on_wait=[w], on_update=[]))
                        nc.register_instruction(nop, overwrite=True)
                        insts.insert(i, nop)
                        i += 1
                i += 1


def build_nc(local_bn=LOCAL_BN, repeat=1):
    nc = bass.Bass(num_devices=N_CORES)
    x_e = nc.declare_dram_parameter("x", [T, 128, 56, 56], BF16, isOutput=False)
    w1_e = nc.declare_dram_parameter("w1", [128, 3 * 24], BF16, isOutput=False)
    d2_e = nc.declare_dram_parameter("d2", [24, 3 * 128], BF16, isOutput=False)
    w3_e = nc.declare_dram_parameter("w3", [128, 3 * 36], BF16, isOutput=False)
    d3a_e = nc.declare_dram_parameter("d3a", [36, 3 * 12], BF16, isOutput=False)
    d3b_e = nc.declare_dram_parameter("d3b", [12, 3 * 128], BF16, isOutput=False)
    b2_e = nc.declare_dram_parameter("b2", [128, 1], F32, isOutput=False)
    b3_e = nc.declare_dram_parameter("b3", [128, 1], F32, isOutput=False)
    gam_e = nc.declare_dram_parameter("gam", [128, 1], F32, isOutput=False)
    bet_e = nc.declare_dram_parameter("bet", [128, 1], F32, isOutput=False)
    out_e = nc.declare_dram_parameter("out", [T, 128, 56, 56], BF16, isOutput=True)

    XLEN = PG + T * PS + PG

    with tile.TileContext(nc) as tc:
        with (
            tc.tile_pool(name="const", bufs=1) as cpool,
            tc.tile_pool(name="xbuf", bufs=1) as xpool,
            tc.tile_pool(name="stg", bufs=2) as spool,      # s2/s3/bnr staging
            tc.tile_pool(name="attn", bufs=2) as apool,
            tc.tile_pool(name="gate", bufs=2) as gpool,
            tc.tile_pool(name="ybuf", bufs=2) as ypool,
            tc.tile_pool(name="psum", bufs=2, space=bass.MemorySpace.PSUM) as psum,
            tc.tile_pool(name="dram", bufs=1, space="DRAM") as dram,
        ):
            # ---- constants ----
            w1 = cpool.tile([128, 3 * 24], BF16)
            d2 = cpool.tile([24, 3 * 128], BF16)
            w3 = cpool.tile([128, 3 * 36], BF16)
            d3a = cpool.tile([36, 3 * 12], BF16)
            d3b = cpool.tile([12, 3 * 128], BF16)
            b2 = cpool.tile([128, 1], F32)
            b3 = cpool.tile([128, 1], F32)
            gam = cpool.tile([128, 1], F32)
            bet = cpool.tile([128, 1], F32)
            for t_, e_ in ((w1, w1_e), (d2, d2_e), (w3, w3_e), (d3a, d3a_e),
                           (d3b, d3b_e), (b2, b2_e), (b3, b3_e),
                           (gam, gam_e), (bet, bet_e)):
                nc.sync.dma_start(t_[:], e_[:])

            X = xpool.tile([128, XLEN], BF16)
            P = xpool.tile([128, 4 * IN], BF16)          # P ring, packed planes
            TR = xpool.tile([12, 4 * PS], BF16)          # t ring
            sums = cpool.tile([128, T], F32)
            sumsqs = cpool.tile([128, T], F32)
            scr = cpool.tile([128, PS], BF16)            # sumsq scratch out
            red = cpool.tile([128, 2], F32)
            st = cpool.tile([64, 8], F32)
            arl = cpool.tile([64, 2], F32)
            scb = cpool.tile([128, 2], F32)
            if local_bn:
                arg = red
            else:
                arg = cpool.tile([128, 2], F32)

            def xbase(u):
                return PG + u * PS

            def xpl(u, off=0, ln=PS):
                return X[:, xbase(u) + off: xbase(u) + off + ln]

            def xint(u, rows=(0, 128)):
                base = xbase(u) + PG
                ap = X[rows[0]:rows[1], base: base + 56 * 58]
                return ap.rearrange("p (h w) -> p h w", h=56)[:, :, 0:56]

            def pflat(u, rows=(0, 128)):
                s = (u % 4) * IN
                return P[rows[0]:rows[1], s: s + IN]

            def pint(u, rows=(0, 128)):
                return pflat(u, rows).rearrange("p (h w) -> p h w", h=56)

            # ---- zero guards / borders (gpsimd memsets are cheap) ----
            nc.gpsimd.memset(X[:, 0:PG], 0.0)
            nc.gpsimd.memset(X[:, XLEN - PG: XLEN], 0.0)
            for u in range(T):
                b = xbase(u)
                nc.gpsimd.memset(X[:, b: b + 58], 0.0)                 # row 0
                nc.gpsimd.memset(X[:, b + 57 * 58: b + PS], 0.0)       # row 57
                ap = X[:, b: b + PS].rearrange("p (h w) -> p h w", h=58)
                nc.gpsimd.memset(ap[:, :, 0:1], 0.0)                   # col 0
                nc.gpsimd.memset(ap[:, :, 57:58], 0.0)                 # col 57

            for rep in range(repeat):
                # ---- input DMA (all 8 frames up front, SP queue) ----
                for u in range(T):
                    nc.sync.dma_start(xint(u), x_e[u])

                # ======= phase A: conv2d attn + gating + stats =======
                for u in range(T):
                    s2 = spool.tile([24, ROWG + PS + ROWG], BF16,
                                    name=f"s2_{rep}_{u}", tag="s2")
                    if rep == 0 and u < 2:
                        nc.gpsimd.memset(s2[:, 0:ROWG], 0.0)
                        nc.gpsimd.memset(s2[:, ROWG + PS:], 0.0)
                    for q in range(4):
                        ps = psum.tile([128, QUAR], F32,
                                       name=f"a1_{rep}_{u}_{q}", tag="small")
                        for ck, ln in SUBS:
                            for dxi in range(3):
                                nc.tensor.matmul(
                                    ps[0:24, ck:ck + ln],
                                    w1[:, dxi * 24:(dxi + 1) * 24],
                                    xpl(u, q * QUAR + ck + dxi - 1, ln),
                                    start=(dxi == 0), stop=(dxi == 2))
                        nc.vector.tensor_copy(
                            s2[0:24, ROWG + q * QUAR: ROWG + (q + 1) * QUAR],
                            ps[0:24, :])
                    at = apool.tile([128, PS], BF16, name=f"attn_{rep}_{u}",
                                    tag="attn")
                    for q in range(4):
                        ps2 = psum.tile([128, QUAR], F32,
                                        name=f"a2_{rep}_{u}_{q}", tag="big")
                        for ck, ln in SUBS:
                            for dyi in range(3):
                                o = ROWG + q * QUAR + ck + 58 * (dyi - 1)
                                nc.tensor.matmul(
                                    ps2[:, ck:ck + ln],
                                    d2[0:24, dyi * 128:(dyi + 1) * 128],
                                    s2[0:24, o: o + ln],
                                    start=(dyi == 0), stop=(dyi == 2))
                        nc.scalar.activation(at[:, q * QUAR:(q + 1) * QUAR],
                                             ps2[:, :], AF.Sigmoid,
                                             bias=b2[:, 0:1])
                    # x2 = max(attn, 0.5) * x  (in place), accumulate sums
                    nc.vector.scalar_tensor_tensor(
                        out=xpl(u), in0=at[:, :], scalar=0.5, in1=xpl(u),
                        op0=ALU.max, op1=ALU.mult, accum_out=sums[:, u:u + 1])
                    nc.scalar.activation(scr[:, :], xpl(u), AF.Square,
                                         accum_out=sumsqs[:, u:u + 1])

                # ---- BN stats ----
                nc.vector.tensor_reduce(red[:, 0:1], sums[:],
                                        mybir.AxisListType.X, ALU.add)
                nc.vector.tensor_reduce(red[:, 1:2], sumsqs[:],
                                        mybir.AxisListType.X, ALU.add)
                if local_bn:
                    ntot = NTOT_LOCAL
                else:
                    cc_in = dram.tile([128, 2], F32, name=f"cc_in_{rep}")
                    cc_out = dram.tile([128, 2], F32, addr_space="Shared",
                                       name=f"cc_out_{rep}")
                    nc.sync.dma_start(cc_in[:], red[:])
                    nc.gpsimd.collective_compute(
                        "AllReduce", ALU.add,
                        replica_groups=[list(range(N_CORES))],
                        ins=[cc_in[:].opt()], outs=[cc_out[:].opt()])
                    nc.sync.dma_start(arg[:], cc_out[:])
                    ntot = NTOT_GLOBAL

                nc.vector.tensor_copy(arl[:, :], arg[64:128, :])
                nc.vector.tensor_tensor(st[:, 0:2], arg[0:64, :], arl[:, :],
                                        ALU.add)
                nc.vector.tensor_scalar_mul(st[:, 2:4], st[:, 0:2], 1.0 / ntot)
                nc.vector.tensor_tensor(st[:, 4:5], st[:, 2:3], st[:, 2:3],
                                        ALU.mult)
                nc.vector.tensor_tensor(st[:, 4:5], st[:, 3:4], st[:, 4:5],
                                        ALU.subtract)
                nc.vector.tensor_scalar_add(st[:, 4:5], st[:, 4:5], EPS)
                # rsqrt: s = sqrt(v); r = 1/s; Newton: r = r*(2 - s*r)
                nc.scalar.activation(st[:, 5:6], st[:, 4:5], AF.Sqrt)
                nc.vector.reciprocal(st[:, 6:7], st[:, 5:6])
                nc.vector.tensor_tensor(st[:, 7:8], st[:, 5:6], st[:, 6:7],
                                        ALU.mult)
                nc.vector.tensor_scalar(st[:, 7:8], st[:, 7:8], -1.0, 2.0,
                                        ALU.mult, ALU.add)
                nc.vector.tensor_tensor(st[:, 6:7], st[:, 6:7], st[:, 7:8],
                                        ALU.mult)
                nc.vector.tensor_tensor(scb[0:64, 0:1], gam[0:64, :],
                                        st[:, 6:7], ALU.mult)
                nc.vector.tensor_tensor(st[:, 7:8], st[:, 2:3], scb[0:64, 0:1],
                                        ALU.mult)
                nc.vector.tensor_tensor(scb[0:64, 1:2], bet[0:64, :],
                                        st[:, 7:8], ALU.subtract)
                nc.vector.tensor_copy(scb[64:128, :], scb[0:64, :])

                # ======= phase B: bnrelu + conv3d + combine =======
                def bnrelu(u):
                    bt = spool.tile([128, PG + PS + PG], BF16,
                                    name=f"bnr_{rep}_{u}", tag="bnr")
                    if rep == 0 and u < 2:
                        nc.gpsimd.memset(bt[:, 0:PG + 58], 0.0)
                        nc.gpsimd.memset(bt[:, PG + 57 * 58:], 0.0)
                        ap = bt[:, PG: PG + PS].rearrange(
                            "p (h w) -> p h w", h=58)
                        nc.gpsimd.memset(ap[:, :, 0:1], 0.0)
                        nc.gpsimd.memset(ap[:, :, 57:58], 0.0)
                    dst = bt[:, PG + PG: PG + PG + 56 * 58].rearrange(
                        "p (h w) -> p h w", h=56)[:, :, 0:56]
                    nc.scalar.activation(dst, xint(u), AF.Relu,
                                         bias=scb[:, 1:2], scale=scb[:, 0:1])
                    return bt

                def conv3d_front(u, bt):
                    s3 = spool.tile([36, ROWG + PS + ROWG], BF16,
                                    name=f"s3_{rep}_{u}", tag="s3")
                    if rep == 0 and u < 2:
                        nc.gpsimd.memset(s3[:, 0:ROWG], 0.0)
                        nc.gpsimd.memset(s3[:, ROWG + PS:], 0.0)
                    for q in range(4):
                        ps = psum.tile([128, QUAR], F32,
                                       name=f"c1_{rep}_{u}_{q}", tag="small")
                        for ck, ln in SUBS:
                            for dxi in range(3):
                                o = PG + q * QUAR + ck + dxi - 1
                                nc.tensor.matmul(
                                    ps[0:36, ck:ck + ln],
                                    w3[:, dxi * 36:(dxi + 1) * 36],
                                    bt[:, o: o + ln],
                                    start=(dxi == 0), stop=(dxi == 2))
                        nc.scalar.activation(
                            s3[0:36, ROWG + q * QUAR: ROWG + (q + 1) * QUAR],
                            ps[0:36, :], AF.Copy)
                    for q in range(4):
                        ps = psum.tile([128, QUAR], F32,
                                       name=f"c2_{rep}_{u}_{q}", tag="small")
                        for ck, ln in SUBS:
                            for dyi in range(3):
                                o = ROWG + q * QUAR + ck + 58 * (dyi - 1)
                                nc.tensor.matmul(
                                    ps[0:12, ck:ck + ln],
                                    d3a[0:36, dyi * 12:(dyi + 1) * 12],
                                    s3[0:36, o: o + ln],
                                    start=(dyi == 0), stop=(dyi == 2))
                        nc.vector.tensor_copy(
                            TR[0:12, (u % 4) * PS + q * QUAR:
                               (u % 4) * PS + (q + 1) * QUAR], ps[0:12, :])

                def conv3d_back(v):
                    dts = [dti for dti in range(3) if 0 <= v + dti - 1 < T]
                    gt = gpool.tile([128, PS], BF16, name=f"gate_{rep}_{v}",
                                    tag="gate")
                    for q in range(4):
                        ps = psum.tile([128, QUAR], F32,
                                       name=f"c3_{rep}_{v}_{q}", tag="big")
                        for ck, ln in SUBS:
                            for i, dti in enumerate(dts):
                                s = ((v + dti - 1) % 4) * PS
                                nc.tensor.matmul(
                                    ps[:, ck:ck + ln],
                                    d3b[0:12, dti * 128:(dti + 1) * 128],
                                    TR[0:12, s + q * QUAR + ck:
                                       s + q * QUAR + ck + ln],
                                    start=(i == 0), stop=(i == len(dts) - 1))
                        nc.scalar.activation(gt[:, q * QUAR:(q + 1) * QUAR],
                                             ps[:, :], AF.Tanh, bias=b3[:, 0:1])
                    gint = gt[:, PG: PG + 56 * 58].rearrange(
                        "p (h w) -> p h w", h=56)[:, :, 0:56]
                    nc.vector.tensor_tensor(pint(v), gint, xint(v), ALU.mult)

                def emit(w):
                    yt = ypool.tile([128, IN], BF16, name=f"y_{rep}_{w}",
                                    tag="y")
                    yv = yt[:].rearrange("p (h w) -> p h w", h=56)
                    nc.vector.tensor_tensor(yv, xint(w), pint(w), ALU.subtract)
                    # g1 (h=0: parts 0:32, 64:96) += P[w+1]; g2 += P[w-1]
                    if w + 1 < T:
                        nc.vector.tensor_tensor(yt[0:32, :], yt[0:32, :],
                                                pflat(w + 1, (0, 32)), ALU.add)
                        nc.gpsimd.tensor_tensor(yt[64:96, :], yt[64:96, :],
                                                pflat(w + 1, (64, 96)),
                                                ALU.add)
                    if w - 1 >= 0:
                        nc.vector.tensor_tensor(yt[32:64, :], yt[32:64, :],
                                                pflat(w - 1, (32, 64)),
                                                ALU.add)
                        nc.gpsimd.tensor_tensor(yt[96:128, :], yt[96:128, :],
                                                pflat(w - 1, (96, 128)),
                                                ALU.add)
                    nc.sync.dma_start(out_e[w], yt[:])

                for u in range(T):
                    bt = bnrelu(u)
                    conv3d_front(u, bt)
                    if u >= 1:
                        conv3d_back(u - 1)
                    if u >= 2:
                        emit(u - 2)
                conv3d_back(T - 1)
                emit(T - 2)
                emit(T - 1)

    return nc


CHMAP = np.array([32 * (q // 32) + 16 * ((q % 32) % 2) + (q % 32) // 2
                  for q in range(64)])


def _host_prep(attn_w, attn_b, bn_gamma, bn_beta, conv3d_w, conv3d_b):
    bf = ml_dtypes.bfloat16
    ch = np.concatenate([CHMAP, CHMAP])          # ch(p), p = 64*clip + q
    clip = np.arange(128) // 64
    hh = (np.arange(128) % 64) // 32             # group/half of partition

    W1 = np.zeros((128, 3 * 24), np.float32)
    for dxi in range(3):
        for dyi in range(3):
            for head in range(4):
                for cl in range(2):
                    col = dxi * 24 + dyi * 8 + head * 2 + cl
                    m = clip == cl
                    W1[m, col] = attn_w[head, ch[m], dyi, dxi]
    D2 = np.zeros((24, 3 * 128), np.float32)
    for dyi in range(3):
        for p in range(128):
            head = ch[p] // 16
            D2[dyi * 8 + head * 2 + clip[p], dyi * 128 + p] = 1.0
    W3 = np.zeros((128, 3 * 36), np.float32)
    for dxi in range(3):
        for dti in range(3):
            for dyi in range(3):
                for cl in range(2):
                    for g in range(2):
                        col = dxi * 36 + dti * 12 + dyi * 4 + g * 2 + cl
                        m = (clip == cl) & (hh == g)
                        W3[m, col] = conv3d_w[g, ch[m] % 32, dti, dyi, dxi]
    D3A = np.zeros((36, 3 * 12), np.float32)
    for dyi in range(3):
        for dti in range(3):
            for g in range(2):
                for cl in range(2):
                    D3A[dti * 12 + dyi * 4 + g * 2 + cl,
                        dyi * 12 + dti * 4 + g * 2 + cl] = 1.0
    D3B = np.zeros((12, 3 * 128), np.float32)
    for dti in range(3):
        for p in range(128):
            D3B[dti * 4 + hh[p] * 2 + clip[p], dti * 128 + p] = 1.0
    b2r = attn_b[ch // 16].astype(np.float32).reshape(128, 1)
    b3r = conv3d_b[hh].astype(np.float32).reshape(128, 1)
    gr = bn_gamma[ch].astype(np.float32).reshape(128, 1)
    br = bn_beta[ch].astype(np.float32).reshape(128, 1)
    return (W1.astype(bf), D2.astype(bf), W3.astype(bf), D3A.astype(bf),
            D3B.astype(bf), b2r, b3r, gr, br)


def get_nc():
    if "nc" not in _CACHE:
        nc = build_nc()
        _legalize_waits(nc)
        _CACHE["nc"] = nc
    return _CACHE["nc"]


def make_in_maps(x, attn_w, attn_b, bn_gamma, bn_beta, conv3d_w, conv3d_b):
    W1, D2, W3, D3A, D3B, b2r, b3r, gr, br = _host_prep(
        np.asarray(attn_w), np.asarray(attn_b), np.asarray(bn_gamma),
        np.asarray(bn_beta), np.asarray(conv3d_w), np.asarray(conv3d_b))
    x = np.asarray(x)
    bf = ml_dtypes.bfloat16
    in_maps = []
    for k in range(N_CORES):
        xk = x[16 * k:16 * (k + 1)].reshape(2, T, 64, 56, 56)
        xp = np.ascontiguousarray(
            xk[:, :, CHMAP].transpose(1, 0, 2, 3, 4).reshape(
                T, 128, 56, 56)).astype(bf)
        in_maps.append({
            "x": xp, "w1": W1, "d2": D2, "w3": W3, "d3a": D3A, "d3b": D3B,
            "b2": b2r, "b3": b3r, "gam": gr, "bet": br,
        })
    return in_maps


def kernel(x, attn_w, attn_b, bn_gamma, bn_beta, conv3d_w, conv3d_b):
    nc = get_nc()
    in_maps = make_in_maps(x, attn_w, attn_b, bn_gamma, bn_beta,
                           conv3d_w, conv3d_b)
    res = run_bass_kernel_spmd(nc, in_maps, core_ids=list(range(N_CORES)))
    parts = []
    for k in range(N_CORES):
        r = np.asarray(res.results[k]["out"]).astype(np.float32)
        parts.append(r.reshape(T, 2, 64, 56, 56).transpose(1, 0, 2, 3, 4)
                      .reshape(16, 64, 56, 56))
    return np.concatenate(parts, 0)
